# revision 10
# baseline (speedup 1.0000x reference)
"""Trainium2 Bass kernel for nn_AdaptiveCouplingLayer (Kuramoto coupling layer).

Self-contained: takes FULL inputs, shards over 8 NeuronCores internally
(data-parallel over batch x sequence-half), returns FULL outputs.

Sharding: core c handles batch b = c//2, tokens s in [4096*(c%2), 4096*(c%2)+4096).
Host prep transposes hidden to [H, S_local] per core so the contraction dim (H)
lands on SBUF partitions; w_out is fed in natural [H, O] layout.
"""
import math
import os

import numpy as np

MIN_K, MAX_K, NUM_STEPS, DT = 0.1, 3.0, 5, 0.01
PI = math.pi

B, S, H = 4, 8192, 1024
HQ = H // 4          # 256
N_CORES = 8
TL = S * B // N_CORES  # 4096 tokens per core
NJ = TL // 128         # 32 t-tiles per core
NK = H // 128          # 8 h-tiles
INV_S = 1.0 / S

_CACHE = {}

LAST_RESULT = None


def _build(scalars):
    """Build + schedule the Bass program. scalars = (b_phase, b_freq, b_c2)."""
    from concourse import bass, bacc, mybir, tile

    F32 = mybir.dt.float32
    F32R = mybir.dt.float32r
    AF = mybir.ActivationFunctionType
    ALU = mybir.AluOpType

    b_phase, b_freq, b_c2 = scalars

    nc = bacc.Bacc("TRN2", target_bir_lowering=False, debug=False,
                   num_devices=N_CORES)

    # ---- DRAM parameters (per-core shards; host packs these) ----
    hidT_ext = nc.declare_dram_parameter("hidT", [H, TL], F32R, isOutput=False)
    w_ext = nc.declare_dram_parameter("w_main", [H, H], F32R, isOutput=False)
    wpib_ext = nc.declare_dram_parameter("w_pi_b", [4, H], F32R, isOutput=False)
    wpf_ext = nc.declare_dram_parameter("wpf", [H, 2], F32, isOutput=False)
    w1_ext = nc.declare_dram_parameter("w_c1", [128, 16 * 128], F32, isOutput=False)
    w2_ext = nc.declare_dram_parameter("w_c2", [128, 2], F32, isOutput=False)
    b1t_ext = nc.declare_dram_parameter("b_c1t", [128, 2], F32, isOutput=False)
    ones_ext = nc.declare_dram_parameter("ones", [128, 128], F32, isOutput=False)
    onesr_ext = nc.declare_dram_parameter("ones_row", [1, TL], F32R, isOutput=False)

    out_ext = nc.declare_dram_parameter("out", [TL, H], F32, isOutput=True)
    r_ext = nc.declare_dram_parameter("r_out", [1, 1], F32, isOutput=True)
    k_ext = nc.declare_dram_parameter("K_out", [1, 1], F32, isOutput=True)

    # collective bounce buffers (pack: ph 0:32 | dtf 32:64 | gctx 64:72)
    agin = nc.dram_tensor("agin", [128, 72], F32)
    agout = nc.dram_tensor("agout", [N_CORES * 128, 72], F32, addr_space="Shared")

    with tile.TileContext(nc) as tc:
        with (
            tc.tile_pool(name="res", bufs=1) as res,
            tc.tile_pool(name="outp", bufs=2) as outp,
            tc.tile_pool(name="psm", bufs=6, space="PSUM") as psm,
            tc.tile_pool(name="pss", bufs=2, space="PSUM") as pss,
        ):
            # ---- resident SBUF tensors ----
            hidT = res.tile([128, NK * TL], F32R, tag="hidT")
            w_sb = res.tile([128, NK * H], F32R, tag="w")
            wpf_sb = res.tile([128, 2 * NK], F32, tag="wpf")
            w1_sb = res.tile([128, 16 * 128], F32, tag="w1")
            w2_sb = res.tile([128, 2], F32, tag="w2")
            b1t_sb = res.tile([128, 2], F32, tag="b1t")
            ones_sb = res.tile([128, 128], F32, tag="ones")
            wcs3_sb = res.tile([3, H], F32R, tag="wcs3")
            wr_sb = res.tile([1, H], F32R, tag="wr")
            b0_sb = res.tile([1, H], F32R, tag="b0")
            wrb_sb = res.tile([1, H], F32R, tag="wrb")
            cs3_sb = res.tile([3, TL], F32R, tag="cs3")
            ph_sb = res.tile([128, 64], F32, tag="ph")
            dtf_sb = res.tile([128, 64], F32, tag="dtf")
            csh_sb = res.tile([128, 64], F32, tag="csh")
            cs_sb = res.tile([128, 128], F32R, tag="cs")
            u1_sb = res.tile([128, 64], F32, tag="u1")
            u2_sb = res.tile([128, 64], F32, tag="u2")
            rsum_sb = res.tile([128, 2], F32, tag="rsum")
            gct_sb = res.tile([128, NK], F32, tag="gct")
            gtmp_sb = res.tile([128, 64], F32, tag="gtmp")
            ptn_sb = res.tile([128, 72], F32, tag="ptn")
            hmid_sb = res.tile([128, 2], F32, tag="hmid")
            small_sb = res.tile([128, 8], F32, tag="small")
            # small_sb cols: 0=r_col, 1=ndtkn_col, 2=sig, 3=K, 4=ndtkn, 5=S2, 6=q

            # ---- constant / weight DMAs ----
            nc.sync.dma_start(
                w_sb[:],
                w_ext[:].rearrange("(k p) o -> p k o", k=NK, p=128),
            )
            nc.sync.dma_start(
                wpf_sb[:],
                wpf_ext[:].rearrange("(k p) c -> p k c", k=NK, p=128),
            )
            nc.sync.dma_start(w1_sb[:], w1_ext[:])
            nc.sync.dma_start(w2_sb[:], w2_ext[:])
            nc.sync.dma_start(b1t_sb[:], b1t_ext[:])
            nc.sync.dma_start(ones_sb[:], ones_ext[:])
            nc.sync.dma_start(wr_sb[:], wpib_ext[2:3, :])
            nc.sync.dma_start(b0_sb[:], wpib_ext[3:4, :])
            # wcs3 rows 0,1 = wc, ws; row 2 = r*wr + b_out (filled later)
            nc.sync.dma_start(wcs3_sb[0:2, :], wpib_ext[0:2, :])
            # cs3 row 2 = ones
            nc.sync.dma_start(cs3_sb[2:3, :], onesr_ext[:])

            # ---- hidden DMAs: j4-outer, k-inner (512-token column chunks) ----
            for j4 in range(8):
                for k in range(NK):
                    nc.sync.dma_start(
                        hidT[:, k * TL + 512 * j4:k * TL + 512 * j4 + 512],
                        hidT_ext[128 * k:128 * k + 128, 512 * j4:512 * j4 + 512],
                    )

            # ---- phase A: per-token phase/freq projections + gctx partials ----
            for j4 in range(8):
                for jj in range(4):
                    j = 4 * j4 + jj
                    pf_ps = pss.tile([128, 2], F32, tag="pf")
                    for k in range(NK):
                        nc.tensor.matmul(
                            pf_ps[:],
                            hidT[:, k * TL + 128 * j:k * TL + 128 * j + 128].bitcast(F32),
                            wpf_sb[:, 2 * k:2 * k + 2],
                            start=(k == 0), stop=(k == NK - 1),
                        )
                    # drain: ph col j = tanh(p_raw + b_phase); dtf col j = DT*(f + b_freq)
                    nc.scalar.activation(ph_sb[:, j:j + 1], pf_ps[:, 0:1],
                                         AF.Tanh, bias=float(b_phase), scale=1.0)
                    nc.vector.tensor_scalar(
                        out=dtf_sb[:, j:j + 1], in0=pf_ps[:, 1:2],
                        scalar1=DT, scalar2=DT * float(b_freq),
                        op0=ALU.mult, op1=ALU.add,
                    )
                # gctx partial per (k, j4) chunk
                for k in range(NK):
                    nc.vector.tensor_reduce(
                        out=gtmp_sb[:, 8 * k + j4:8 * k + j4 + 1],
                        in_=hidT[:, k * TL + 512 * j4:k * TL + 512 * j4 + 512].bitcast(F32),
                        axis=mybir.AxisListType.X, op=ALU.add,
                    )
            # finish gctx partials: [128, 8] per-core sums over own tokens
            for k in range(NK):
                nc.vector.tensor_reduce(
                    out=gct_sb[:, k:k + 1],
                    in_=gtmp_sb[:, 8 * k:8 * k + 8],
                    axis=mybir.AxisListType.X, op=ALU.add,
                )
            # phases *= pi (own half, cols 0:32)
            nc.vector.tensor_single_scalar(
                out=ph_sb[:, 0:32], in_=ph_sb[:, 0:32], scalar=PI, op=ALU.mult)

            # ---- AllGather: share (ph, dtf, gctx) with partner ----
            nc.sync.dma_start(agin[:, 0:32], ph_sb[:, 0:32])
            nc.sync.dma_start(agin[:, 32:64], dtf_sb[:, 0:32])
            nc.sync.dma_start(agin[:, 64:72], gct_sb[:])
            nc.gpsimd.collective_compute(
                "AllGather", ALU.bypass,
                ins=[agin[:].opt()], outs=[agout[:].opt()],
                replica_groups=[list(range(N_CORES))],
            )
            pid = nc.sync.partition_id()
            for c in range(N_CORES):
                t = c ^ 1
                is_partner = (pid >= t) & (pid <= t)
                nc.sync.dma_start(ptn_sb[:], agout[128 * c:128 * c + 128, :],
                                  cond=is_partner)
            nc.vector.tensor_copy(ph_sb[:, 32:64], ptn_sb[:, 0:32])
            nc.vector.tensor_copy(dtf_sb[:, 32:64], ptn_sb[:, 32:64])
            nc.vector.tensor_tensor(out=gct_sb[:], in0=gct_sb[:],
                                    in1=ptn_sb[:, 64:72], op=ALU.add)

            # ---- K MLP: gctx -> gelu(W1) -> sigmoid(W2) -> K ----
            hmid_ps = pss.tile([128, 2], F32, tag="pf")
            for q in range(2):
                for k in range(NK):
                    nc.tensor.matmul(
                        hmid_ps[:, q:q + 1],
                        w1_sb[:, (2 * k + q) * 128:(2 * k + q) * 128 + 128],
                        gct_sb[:, k:k + 1],
                        start=(k == 0), stop=(k == NK - 1),
                    )
            for q in range(2):
                nc.scalar.activation(hmid_sb[:, q:q + 1], hmid_ps[:, q:q + 1],
                                     AF.Gelu, bias=b1t_sb[:, q:q + 1], scale=INV_S)
            kp_ps = pss.tile([1, 1], F32, tag="pf")
            for q in range(2):
                nc.tensor.matmul(
                    kp_ps[:],
                    hmid_sb[:, q:q + 1],
                    w2_sb[:, q:q + 1],
                    start=(q == 0), stop=(q == 1),
                )
            nc.scalar.activation(small_sb[0:1, 2:3], kp_ps[:], AF.Sigmoid,
                                 bias=float(b_c2), scale=1.0)
            # K = MIN_K + (MAX_K-MIN_K)*sig ; ndtkn = -DT*K/S
            nc.vector.tensor_scalar(
                out=small_sb[0:1, 3:4], in0=small_sb[0:1, 2:3],
                scalar1=MAX_K - MIN_K, scalar2=MIN_K, op0=ALU.mult, op1=ALU.add)
            nc.vector.tensor_scalar(
                out=small_sb[0:1, 4:5], in0=small_sb[0:1, 2:3],
                scalar1=-DT * (MAX_K - MIN_K) * INV_S,
                scalar2=-DT * MIN_K * INV_S, op0=ALU.mult, op1=ALU.add)
            nc.sync.dma_start(k_ext[:], small_sb[0:1, 3:4])
            # replicate ndtkn to all partitions: ones[1,128].T @ ndtkn[1,1]
            rep_ps = pss.tile([128, 1], F32, tag="pf")
            nc.tensor.matmul(rep_ps[:], ones_sb[0:1, 0:128], small_sb[0:1, 4:5],
                             start=True, stop=True)
            nc.vector.tensor_copy(small_sb[:, 1:2], rep_ps[:])

            # ---- Kuramoto steps on [128, 64] (8192 tokens of this batch) ----
            for step in range(NUM_STEPS + 1):
                last = step == NUM_STEPS
                # c = sin(wrap(ph + pi/2)), s = sin(ph); rowsums via accum
                nc.vector.add_range_wrap(csh_sb[:], ph_sb[:], PI / 2, PI, 2 * PI)
                with nc.allow_low_precision(reason="f32r cs for matmul epilogue"):
                    nc.scalar.activation(cs_sb[:, 0:64], csh_sb[:], AF.Sin,
                                         accum_out=rsum_sb[:, 0:1])
                    nc.scalar.activation(cs_sb[:, 64:128], ph_sb[:], AF.Sin,
                                         accum_out=rsum_sb[:, 1:2])
                sums_ps = pss.tile([128, 2], F32, tag="pf")
                nc.tensor.matmul(sums_ps[:], ones_sb[:], rsum_sb[:],
                                 start=True, stop=True)
                if not last:
                    # u1 = s*C ; u2 = c*S - u1 = -interaction
                    nc.vector.tensor_single_scalar(
                        out=u1_sb[:], in_=cs_sb[:, 64:128].bitcast(F32),
                        scalar=sums_ps[:, 0:1], op=ALU.mult)
                    nc.vector.scalar_tensor_tensor(
                        out=u2_sb[:], in0=cs_sb[:, 0:64].bitcast(F32),
                        scalar=sums_ps[:, 1:2], in1=u1_sb[:],
                        op0=ALU.mult, op1=ALU.subtract)
                    # ph = wrap(ph + dtf + ndtkn*u2)
                    nc.vector.tensor_tensor(out=ph_sb[:], in0=ph_sb[:],
                                            in1=dtf_sb[:], op=ALU.add)
                    nc.vector.scalar_tensor_tensor(
                        out=ph_sb[:], in0=u2_sb[:], scalar=small_sb[:, 1:2],
                        in1=ph_sb[:], op0=ALU.mult, op1=ALU.add)
                    nc.vector.add_range_wrap(ph_sb[:], ph_sb[:], 0.0, PI, 2 * PI)
                else:
                    # r = sqrt((C/S)^2 + (Ssum/S)^2) replicated
                    nc.vector.tensor_single_scalar(
                        out=small_sb[:, 5:6], in_=sums_ps[:, 1:2],
                        scalar=sums_ps[:, 1:2], op=ALU.mult)
                    nc.vector.scalar_tensor_tensor(
                        out=small_sb[:, 6:7], in0=sums_ps[:, 0:1],
                        scalar=sums_ps[:, 0:1], in1=small_sb[:, 5:6],
                        op0=ALU.mult, op1=ALU.add)
                    nc.scalar.activation(small_sb[:, 0:1], small_sb[:, 6:7],
                                         AF.Sqrt, bias=0.0, scale=INV_S * INV_S)
                    nc.sync.dma_start(r_ext[:], small_sb[0:1, 0:1])

            # ---- build cs3 rows (c_own, s_own) in p-major order + wrb row ----
            # row element e = p*NJ + j  <->  token t = 128*j + p
            # cs_sb own half = cols 0:32 (c), 64:96 (s)
            nc.sync.dma_start(
                cs3_sb[0:1, :].rearrange("a (p j) -> a p j", p=128, j=NJ),
                cs_sb[:, 0:32],
            )
            nc.sync.dma_start(
                cs3_sb[1:2, :].rearrange("a (p j) -> a p j", p=128, j=NJ),
                cs_sb[:, 64:96],
            )
            # wrb = r*wr + b_out (on partition 0), then DMA into wcs3 row 2
            with nc.allow_low_precision(reason="f32r epilogue row"):
                nc.vector.scalar_tensor_tensor(
                    out=wrb_sb[0:1, :], in0=wr_sb[0:1, :].bitcast(F32),
                    scalar=small_sb[0:1, 0:1], in1=b0_sb[0:1, :].bitcast(F32),
                    op0=ALU.mult, op1=ALU.add)
            nc.sync.dma_start(wcs3_sb[2:3, :], wrb_sb[0:1, :])

            # ---- main matmul + fused phase epilogue ----
            # cs3 strided view: [3, j, p] with token t = 128*j + p
            cs3_v = cs3_sb[:].rearrange("a (p j) -> a j p", p=128, j=NJ)
            for j in range(NJ):
                for half in range(2):
                    o0 = 512 * half
                    mm_ps = psm.tile([128, 512], F32, tag="mm")
                    for k in range(NK):
                        nc.tensor.matmul(
                            mm_ps[:],
                            hidT[:, k * TL + 128 * j:k * TL + 128 * j + 128],
                            w_sb[:, k * H + o0:k * H + o0 + 512],
                            start=(k == 0), stop=False,
                        )
                    nc.tensor.matmul(
                        mm_ps[:],
                        cs3_v[:, j:j + 1, :],
                        wcs3_sb[:, o0:o0 + 512],
                        start=False, stop=True,
                    )
                    ot = outp.tile([128, 512], F32, tag="ot")
                    nc.any.tensor_copy(ot[:], mm_ps[:])
                    nc.sync.dma_start(
                        out_ext[128 * j:128 * j + 128, o0:o0 + 512], ot[:])

    nc.compile()
    return nc


def _get_nc(scalars):
    key = tuple(float(x) for x in scalars)
    if key not in _CACHE:
        _CACHE[key] = _build(key)
    return _CACHE[key]


def kernel(hidden_states, w_c1, b_c1, w_c2, b_c2, w_phase, b_phase,
           w_freq, b_freq, w_out, b_out):
    global LAST_RESULT
    from concourse.bass_utils import run_bass_kernel_spmd

    hidden_states = np.asarray(hidden_states, dtype=np.float32)
    w_c1 = np.asarray(w_c1, dtype=np.float32)
    b_c1 = np.asarray(b_c1, dtype=np.float32)
    w_c2 = np.asarray(w_c2, dtype=np.float32)
    b_c2 = np.asarray(b_c2, dtype=np.float32)
    w_phase = np.asarray(w_phase, dtype=np.float32)
    b_phase = np.asarray(b_phase, dtype=np.float32)
    w_freq = np.asarray(w_freq, dtype=np.float32)
    b_freq = np.asarray(b_freq, dtype=np.float32)
    w_out = np.asarray(w_out, dtype=np.float32)
    b_out = np.asarray(b_out, dtype=np.float32)

    nc = _get_nc((float(b_phase[0]), float(b_freq[0]), float(b_c2[0])))

    # ---- host-side packing (sharding + layout) ----
    w_main = np.ascontiguousarray(w_out[0:H, :])                       # [H, H]
    w_pi_b = np.ascontiguousarray(
        np.concatenate([w_out[H:H + 3, :], b_out[None, :]], axis=0))   # [4, H]
    wpf = np.ascontiguousarray(
        np.concatenate([w_phase, w_freq], axis=1))                     # [H, 2]
    w1 = np.ascontiguousarray(
        w_c1.reshape(NK, 128, 2, 128).transpose(1, 0, 2, 3).reshape(128, 16 * 128))
    w2 = np.ascontiguousarray(w_c2.reshape(2, 128, 1)[:, :, 0].T)      # [128, 2]
    b1t = np.ascontiguousarray(b_c1.reshape(2, 128).T)                 # [128, 2]
    ones = np.ones((128, 128), dtype=np.float32)
    ones_row = np.ones((1, TL), dtype=np.float32)

    in_maps = []
    for c in range(N_CORES):
        b = c // 2
        s0 = TL * (c % 2)
        hidT = np.ascontiguousarray(hidden_states[b, s0:s0 + TL, :].T)  # [H, TL]
        in_maps.append({
            "hidT": hidT, "w_main": w_main, "w_pi_b": w_pi_b, "wpf": wpf,
            "w_c1": w1, "w_c2": w2, "b_c1t": b1t, "ones": ones,
            "ones_row": ones_row,
        })

    res = run_bass_kernel_spmd(nc, in_maps, list(range(N_CORES)),
                               trace=os.environ.get("BASS_TRACE", "0") == "1")
    LAST_RESULT = res

    output = np.empty((B, S, H), dtype=np.float32)
    r = np.empty((B,), dtype=np.float32)
    K = np.empty((B,), dtype=np.float32)
    for c in range(N_CORES):
        b = c // 2
        s0 = TL * (c % 2)
        output[b, s0:s0 + TL, :] = res.results[c]["out"]
        if c % 2 == 0:
            r[b] = res.results[c]["r_out"][0, 0]
            K[b] = res.results[c]["K_out"][0, 0]
    return output, r, K


# revision 14
# speedup vs baseline: 1.1828x; 1.1828x over previous
"""Trainium2 Bass kernel for nn_AdaptiveCouplingLayer (Kuramoto coupling layer).

Self-contained: takes FULL inputs, shards over 8 NeuronCores internally
(data-parallel over batch x sequence-half), returns FULL outputs.

Sharding: core c handles batch b = c//2, tokens s in [4096*(c%2), 4096*(c%2)+4096).
Host prep transposes hidden to [H, S_local] per core so the contraction dim (H)
lands on SBUF partitions; w_out is fed in natural [H, O] layout.

Within a core, local token index t maps to SBUF [128, 32] position
(p, j) with t = 32*p + j for the Kuramoto state, and the epilogue
"cs3" rows hold c/s/1 at row element e = t (natural order).
"""
import math
import os

import numpy as np

MIN_K, MAX_K, NUM_STEPS, DT = 0.1, 3.0, 5, 0.01
PI = math.pi

B, S, H = 4, 8192, 1024
N_CORES = 8
TL = S * B // N_CORES  # 4096 tokens per core
NJ = TL // 128         # 32 t-tiles per core
NK = H // 128          # 8 h-tiles (contraction)
INV_S = 1.0 / S

_CACHE = {}

LAST_RESULT = None


def _build(scalars):
    """Build + schedule the Bass program. scalars = (b_phase, b_freq, b_c2)."""
    from concourse import bass, bacc, mybir, tile

    F32 = mybir.dt.float32
    F32R = mybir.dt.float32r
    AF = mybir.ActivationFunctionType
    ALU = mybir.AluOpType

    b_phase, b_freq, b_c2 = scalars

    nc = bacc.Bacc("TRN2", target_bir_lowering=False, debug=False,
                   num_devices=N_CORES)

    # ---- DRAM parameters (per-core shards; host packs these) ----
    hidT_ext = nc.declare_dram_parameter("hidT", [H, TL], F32R, isOutput=False)
    w_ext = nc.declare_dram_parameter("w_main", [H, H], F32R, isOutput=False)
    wpib_ext = nc.declare_dram_parameter("w_pi_b", [4, H], F32R, isOutput=False)
    wpf_ext = nc.declare_dram_parameter("wpf", [H, 2], F32R, isOutput=False)
    w1_ext = nc.declare_dram_parameter("w_c1", [128, 16 * 128], F32, isOutput=False)
    w2_ext = nc.declare_dram_parameter("w_c2", [128, 2], F32, isOutput=False)
    b1t_ext = nc.declare_dram_parameter("b_c1t", [128, 2], F32, isOutput=False)
    ones_ext = nc.declare_dram_parameter("ones", [128, 128], F32, isOutput=False)
    onesr_ext = nc.declare_dram_parameter("ones_row", [1, TL], F32R, isOutput=False)

    out_ext = nc.declare_dram_parameter("out", [TL, H], F32, isOutput=True)
    r_ext = nc.declare_dram_parameter("r_out", [1, 1], F32, isOutput=True)
    k_ext = nc.declare_dram_parameter("K_out", [1, 1], F32, isOutput=True)

    # collective bounce buffers (pack: ph 0:32 | dtf 32:64 | gctx 64:72)
    agin = nc.dram_tensor("agin", [128, 72], F32)
    agout = nc.dram_tensor("agout", [N_CORES * 128, 72], F32, addr_space="Shared")

    with tile.TileContext(nc) as tc:
        with (
            tc.tile_pool(name="res", bufs=1) as res,
            tc.tile_pool(name="psm", bufs=6, space="PSUM") as psm,
            tc.tile_pool(name="pss", bufs=2, space="PSUM") as pss,
        ):
            # ---- resident SBUF tensors ----
            hidT = res.tile([128, NK * TL], F32R, tag="hidT")
            w_sb = res.tile([128, NK * H], F32R, tag="w")
            wpf_sb = res.tile([128, 2 * NK], F32R, tag="wpf")
            w1_sb = res.tile([128, 16 * 128], F32, tag="w1")
            w2_sb = res.tile([128, 2], F32, tag="w2")
            b1t_sb = res.tile([128, 2], F32, tag="b1t")
            ones_sb = res.tile([128, 128], F32, tag="ones")
            wcs3_sb = res.tile([3, H], F32R, tag="wcs3")
            wr_sb = res.tile([1, H], F32R, tag="wr")
            b0_sb = res.tile([1, H], F32R, tag="b0")
            wrb_sb = res.tile([1, H], F32R, tag="wrb")
            cs3_sb = res.tile([3, TL], F32R, tag="cs3")
            ph_sb = res.tile([128, 64], F32, tag="ph")
            dtf_sb = res.tile([128, 64], F32, tag="dtf")
            csh_sb = res.tile([128, 64], F32, tag="csh")
            cs_sb = res.tile([128, 128], F32R, tag="cs")
            u1_sb = res.tile([128, 64], F32, tag="u1")
            u2_sb = res.tile([128, 64], F32, tag="u2")
            rsum_sb = res.tile([128, 2], F32, tag="rsum")
            gct_sb = res.tile([128, NK], F32, tag="gct")
            gtmp_sb = res.tile([128, 64], F32, tag="gtmp")
            ptn_sb = res.tile([128, 72], F32, tag="ptn")
            hmid_sb = res.tile([128, 2], F32, tag="hmid")
            small_sb = res.tile([128, 8], F32, tag="small")
            # small_sb cols: 0=r_col, 1=ndtkn_col; row0: 2=sig, 3=K, 4=ndtkn
            # cols 5,6: scratch for r

            # ---- hidden DMAs first (j4-outer, k-inner); w after first block ----
            for j4 in range(8):
                for k in range(NK):
                    nc.sync.dma_start(
                        hidT[:, k * TL + 512 * j4:k * TL + 512 * j4 + 512],
                        hidT_ext[128 * k:128 * k + 128, 512 * j4:512 * j4 + 512],
                    )
                if j4 == 0:
                    for k in range(NK):
                        nc.sync.dma_start(
                            w_sb[:, k * H:(k + 1) * H],
                            w_ext[128 * k:128 * k + 128, :],
                        )

            # ---- small constant DMAs ----
            nc.sync.dma_start(
                wpf_sb[:],
                wpf_ext[:].rearrange("(k p) c -> p k c", k=NK, p=128),
            )
            nc.sync.dma_start(w1_sb[:], w1_ext[:])
            nc.sync.dma_start(w2_sb[:], w2_ext[:])
            nc.sync.dma_start(b1t_sb[:], b1t_ext[:])
            nc.sync.dma_start(ones_sb[:], ones_ext[:])
            nc.sync.dma_start(wr_sb[:], wpib_ext[2:3, :])
            nc.sync.dma_start(b0_sb[:], wpib_ext[3:4, :])
            nc.sync.dma_start(wcs3_sb[0:2, :], wpib_ext[0:2, :])
            nc.sync.dma_start(cs3_sb[2:3, :], onesr_ext[:])

            # ---- phase A: p_raw/f_raw rows via w-stationary matmuls ----
            # psum [2, 512] per 512-token chunk; ACT-copy into cs3 rows 0/1
            # (scratch reuse), then shuffle to [128, 32] (t = 32p + j).
            for tc8 in range(8):
                pf_ps = pss.tile([2, 512], F32, tag="pf")
                for k in range(NK):
                    nc.tensor.matmul(
                        pf_ps[:],
                        wpf_sb[:, 2 * k:2 * k + 2],
                        hidT[:, k * TL + 512 * tc8:k * TL + 512 * tc8 + 512],
                        start=(k == 0), stop=(k == NK - 1),
                    )
                with nc.allow_low_precision(reason="f32r pf rows"):
                    nc.scalar.activation(
                        cs3_sb[0:2, 512 * tc8:512 * tc8 + 512],
                        pf_ps[:], AF.Copy)
                # gctx partials for this chunk column range
                for k in range(NK):
                    nc.vector.tensor_reduce(
                        out=gtmp_sb[:, 8 * k + tc8:8 * k + tc8 + 1],
                        in_=hidT[:, k * TL + 512 * tc8:k * TL + 512 * tc8 + 512].bitcast(F32),
                        axis=mybir.AxisListType.X, op=ALU.add,
                    )
            for k in range(NK):
                nc.vector.tensor_reduce(
                    out=gct_sb[:, k:k + 1],
                    in_=gtmp_sb[:, 8 * k:8 * k + 8],
                    axis=mybir.AxisListType.X, op=ALU.add,
                )
            # shuffle p_raw/f_raw rows -> [128, 32] (contiguous: e = 32p + j)
            nc.sync.dma_start(
                ph_sb[:, 0:32],
                cs3_sb[0:1, :].bitcast(F32).rearrange("a (p j) -> a p j", p=128, j=32),
            )
            nc.sync.dma_start(
                dtf_sb[:, 0:32],
                cs3_sb[1:2, :].bitcast(F32).rearrange("a (p j) -> a p j", p=128, j=32),
            )
            # ph = pi * tanh(p_raw + b_phase) ; dtf = DT*(f_raw + b_freq)
            nc.scalar.activation(ph_sb[:, 0:32], ph_sb[:, 0:32], AF.Tanh,
                                 bias=float(b_phase), scale=1.0)
            nc.vector.tensor_single_scalar(
                out=ph_sb[:, 0:32], in_=ph_sb[:, 0:32], scalar=PI, op=ALU.mult)
            nc.vector.tensor_scalar(
                out=dtf_sb[:, 0:32], in0=dtf_sb[:, 0:32],
                scalar1=DT, scalar2=DT * float(b_freq),
                op0=ALU.mult, op1=ALU.add)

            # ---- AllGather: share (ph, dtf, gctx) with partner ----
            nc.sync.dma_start(agin[:, 0:32], ph_sb[:, 0:32])
            nc.sync.dma_start(agin[:, 32:64], dtf_sb[:, 0:32])
            nc.sync.dma_start(agin[:, 64:72], gct_sb[:])
            nc.gpsimd.collective_compute(
                "AllGather", ALU.bypass,
                ins=[agin[:].opt()], outs=[agout[:].opt()],
                replica_groups=[list(range(N_CORES))],
            )
            pid = nc.sync.partition_id()
            for c in range(N_CORES):
                t = c ^ 1
                is_partner = (pid >= t) & (pid <= t)
                nc.sync.dma_start(ptn_sb[:], agout[128 * c:128 * c + 128, :],
                                  cond=is_partner)
            nc.vector.tensor_copy(ph_sb[:, 32:64], ptn_sb[:, 0:32])
            nc.vector.tensor_copy(dtf_sb[:, 32:64], ptn_sb[:, 32:64])
            nc.vector.tensor_tensor(out=gct_sb[:], in0=gct_sb[:],
                                    in1=ptn_sb[:, 64:72], op=ALU.add)

            # ---- K MLP: gctx -> gelu(W1) -> sigmoid(W2) -> K ----
            hmid_ps = pss.tile([128, 2], F32, tag="pf")
            for q in range(2):
                for k in range(NK):
                    nc.tensor.matmul(
                        hmid_ps[:, q:q + 1],
                        w1_sb[:, (2 * k + q) * 128:(2 * k + q) * 128 + 128],
                        gct_sb[:, k:k + 1],
                        start=(k == 0), stop=(k == NK - 1),
                    )
            for q in range(2):
                nc.scalar.activation(hmid_sb[:, q:q + 1], hmid_ps[:, q:q + 1],
                                     AF.Gelu, bias=b1t_sb[:, q:q + 1], scale=INV_S)
            kp_ps = pss.tile([1, 1], F32, tag="pf")
            for q in range(2):
                nc.tensor.matmul(
                    kp_ps[:],
                    hmid_sb[:, q:q + 1],
                    w2_sb[:, q:q + 1],
                    start=(q == 0), stop=(q == 1),
                )
            nc.scalar.activation(small_sb[0:1, 2:3], kp_ps[:], AF.Sigmoid,
                                 bias=float(b_c2), scale=1.0)
            # K = MIN_K + (MAX_K-MIN_K)*sig ; ndtkn = -DT*K/S
            nc.vector.tensor_scalar(
                out=small_sb[0:1, 3:4], in0=small_sb[0:1, 2:3],
                scalar1=MAX_K - MIN_K, scalar2=MIN_K, op0=ALU.mult, op1=ALU.add)
            nc.vector.tensor_scalar(
                out=small_sb[0:1, 4:5], in0=small_sb[0:1, 2:3],
                scalar1=-DT * (MAX_K - MIN_K) * INV_S,
                scalar2=-DT * MIN_K * INV_S, op0=ALU.mult, op1=ALU.add)
            nc.sync.dma_start(k_ext[:], small_sb[0:1, 3:4])
            # replicate ndtkn to all partitions: ones[1,128].T @ ndtkn[1,1]
            rep_ps = pss.tile([128, 1], F32, tag="pf")
            nc.tensor.matmul(rep_ps[:], ones_sb[0:1, 0:128], small_sb[0:1, 4:5],
                             start=True, stop=True)
            nc.vector.tensor_copy(small_sb[:, 1:2], rep_ps[:])

            # ---- Kuramoto steps on [128, 64] (8192 tokens of this batch) ----
            for step in range(NUM_STEPS + 1):
                last = step == NUM_STEPS
                nc.vector.add_range_wrap(csh_sb[:], ph_sb[:], PI / 2, PI, 2 * PI)
                with nc.allow_low_precision(reason="f32r cs for matmul epilogue"):
                    nc.scalar.activation(cs_sb[:, 0:64], csh_sb[:], AF.Sin,
                                         accum_out=rsum_sb[:, 0:1])
                    nc.scalar.activation(cs_sb[:, 64:128], ph_sb[:], AF.Sin,
                                         accum_out=rsum_sb[:, 1:2])
                sums_ps = pss.tile([128, 2], F32, tag="pf")
                nc.tensor.matmul(sums_ps[:], ones_sb[:], rsum_sb[:],
                                 start=True, stop=True)
                if not last:
                    # u1 = s*C ; u2 = c*S - u1 = -interaction
                    nc.vector.tensor_single_scalar(
                        out=u1_sb[:], in_=cs_sb[:, 64:128].bitcast(F32),
                        scalar=sums_ps[:, 0:1], op=ALU.mult)
                    nc.vector.scalar_tensor_tensor(
                        out=u2_sb[:], in0=cs_sb[:, 0:64].bitcast(F32),
                        scalar=sums_ps[:, 1:2], in1=u1_sb[:],
                        op0=ALU.mult, op1=ALU.subtract)
                    # ph = wrap(ph + dtf + ndtkn*u2)
                    nc.vector.tensor_tensor(out=ph_sb[:], in0=ph_sb[:],
                                            in1=dtf_sb[:], op=ALU.add)
                    nc.vector.scalar_tensor_tensor(
                        out=ph_sb[:], in0=u2_sb[:], scalar=small_sb[:, 1:2],
                        in1=ph_sb[:], op0=ALU.mult, op1=ALU.add)
                    nc.vector.add_range_wrap(ph_sb[:], ph_sb[:], 0.0, PI, 2 * PI)
                else:
                    # r = sqrt((Csum/S)^2 + (Ssum/S)^2) replicated
                    nc.vector.tensor_single_scalar(
                        out=small_sb[:, 5:6], in_=sums_ps[:, 1:2],
                        scalar=sums_ps[:, 1:2], op=ALU.mult)
                    nc.vector.scalar_tensor_tensor(
                        out=small_sb[:, 6:7], in0=sums_ps[:, 0:1],
                        scalar=sums_ps[:, 0:1], in1=small_sb[:, 5:6],
                        op0=ALU.mult, op1=ALU.add)
                    nc.scalar.activation(small_sb[:, 0:1], small_sb[:, 6:7],
                                         AF.Sqrt, bias=0.0, scale=INV_S * INV_S)
                    nc.sync.dma_start(r_ext[:], small_sb[0:1, 0:1])

            # ---- build cs3 rows: c_own/s_own as [1, 4096] (e = t = 32p + j) ----
            nc.sync.dma_start(
                cs3_sb[0:1, :].rearrange("a (p j) -> a p j", p=128, j=32),
                cs_sb[:, 0:32],
            )
            nc.sync.dma_start(
                cs3_sb[1:2, :].rearrange("a (p j) -> a p j", p=128, j=32),
                cs_sb[:, 64:96],
            )
            # wrb = r*wr + b_out (partition 0) -> wcs3 row 2
            with nc.allow_low_precision(reason="f32r epilogue row"):
                nc.vector.scalar_tensor_tensor(
                    out=wrb_sb[0:1, :], in0=wr_sb[0:1, :].bitcast(F32),
                    scalar=small_sb[0:1, 0:1], in1=b0_sb[0:1, :].bitcast(F32),
                    op0=ALU.mult, op1=ALU.add)
            nc.sync.dma_start(wcs3_sb[2:3, :], wrb_sb[0:1, :])

            # hidT strip view: [128, k(NK), j(NJ), t(128)]
            hid4 = hidT[:].rearrange(
                "p (k j t) -> p k j t", k=NK, j=NJ, t=128)

            def staged(j, half):
                # 4 strips k = 4*half .. 4*half+3 of tile column j
                lo = 4 * half
                return hid4[:, lo:lo + 4, j:j + 1, :]

            # ---- main matmul: per (j, half) 8-MM group, staged into the ----
            # ---- hidT column strips freed by consuming tile j           ----
            for j in range(NJ):
                for half in range(2):
                    o0 = 512 * half
                    mm_ps = psm.tile([128, 512], F32, tag="mm")
                    for k in range(NK):
                        nc.tensor.matmul(
                            mm_ps[:],
                            hidT[:, k * TL + 128 * j:k * TL + 128 * j + 128],
                            w_sb[:, k * H + o0:k * H + o0 + 512],
                            start=(k == 0), stop=(k == NK - 1),
                        )
                    with nc.allow_low_precision(reason="f32r staging"):
                        nc.any.tensor_copy(staged(j, half), mm_ps[:])

            # ---- phase epilogue: K=3 matmul + add, then DMA out ----
            for j in range(NJ):
                for half in range(2):
                    o0 = 512 * half
                    k3_ps = psm.tile([128, 512], F32, tag="mm")
                    nc.tensor.matmul(
                        k3_ps[:],
                        cs3_sb[:, 128 * j:128 * j + 128],
                        wcs3_sb[:, o0:o0 + 512],
                        start=True, stop=True,
                    )
                    with nc.allow_low_precision(reason="f32r staging add"):
                        nc.vector.tensor_tensor(
                            out=staged(j, half), in0=staged(j, half).bitcast(F32),
                            in1=k3_ps[:], op=ALU.add)
                nc.sync.dma_start(
                    out_ext[128 * j:128 * j + 128, :],
                    hid4[:, 0:NK, j:j + 1, :].bitcast(F32),
                )

    nc.compile()
    return nc


def _get_nc(scalars):
    key = tuple(float(x) for x in scalars)
    if key not in _CACHE:
        _CACHE[key] = _build(key)
    return _CACHE[key]


def kernel(hidden_states, w_c1, b_c1, w_c2, b_c2, w_phase, b_phase,
           w_freq, b_freq, w_out, b_out):
    global LAST_RESULT
    from concourse.bass_utils import run_bass_kernel_spmd

    hidden_states = np.asarray(hidden_states, dtype=np.float32)
    w_c1 = np.asarray(w_c1, dtype=np.float32)
    b_c1 = np.asarray(b_c1, dtype=np.float32)
    w_c2 = np.asarray(w_c2, dtype=np.float32)
    b_c2 = np.asarray(b_c2, dtype=np.float32)
    w_phase = np.asarray(w_phase, dtype=np.float32)
    b_phase = np.asarray(b_phase, dtype=np.float32)
    w_freq = np.asarray(w_freq, dtype=np.float32)
    b_freq = np.asarray(b_freq, dtype=np.float32)
    w_out = np.asarray(w_out, dtype=np.float32)
    b_out = np.asarray(b_out, dtype=np.float32)

    nc = _get_nc((float(b_phase[0]), float(b_freq[0]), float(b_c2[0])))

    # ---- host-side packing (sharding + layout) ----
    w_main = np.ascontiguousarray(w_out[0:H, :])                       # [H, H]
    w_pi_b = np.ascontiguousarray(
        np.concatenate([w_out[H:H + 3, :], b_out[None, :]], axis=0))   # [4, H]
    wpf = np.ascontiguousarray(
        np.concatenate([w_phase, w_freq], axis=1))                     # [H, 2]
    w1 = np.ascontiguousarray(
        w_c1.reshape(NK, 128, 2, 128).transpose(1, 0, 2, 3).reshape(128, 16 * 128))
    w2 = np.ascontiguousarray(w_c2.reshape(2, 128, 1)[:, :, 0].T)      # [128, 2]
    b1t = np.ascontiguousarray(b_c1.reshape(2, 128).T)                 # [128, 2]
    ones = np.ones((128, 128), dtype=np.float32)
    ones_row = np.ones((1, TL), dtype=np.float32)

    in_maps = []
    for c in range(N_CORES):
        b = c // 2
        s0 = TL * (c % 2)
        hidT = np.ascontiguousarray(hidden_states[b, s0:s0 + TL, :].T)  # [H, TL]
        in_maps.append({
            "hidT": hidT, "w_main": w_main, "w_pi_b": w_pi_b, "wpf": wpf,
            "w_c1": w1, "w_c2": w2, "b_c1t": b1t, "ones": ones,
            "ones_row": ones_row,
        })

    res = run_bass_kernel_spmd(nc, in_maps, list(range(N_CORES)),
                               trace=os.environ.get("BASS_TRACE", "0") == "1")
    LAST_RESULT = res

    output = np.empty((B, S, H), dtype=np.float32)
    r = np.empty((B,), dtype=np.float32)
    K = np.empty((B,), dtype=np.float32)
    for c in range(N_CORES):
        b = c // 2
        s0 = TL * (c % 2)
        output[b, s0:s0 + TL, :] = res.results[c]["out"]
        if c % 2 == 0:
            r[b] = res.results[c]["r_out"][0, 0]
            K[b] = res.results[c]["K_out"][0, 0]
    return output, r, K


# revision 15
# speedup vs baseline: 1.3213x; 1.1172x over previous
"""Trainium2 Bass kernel for nn_AdaptiveCouplingLayer (Kuramoto coupling layer).

Self-contained: takes FULL inputs, shards over 8 NeuronCores internally
(data-parallel over batch x sequence-half), returns FULL outputs.

Sharding: core c handles batch b = c//2, tokens s in [4096*(c%2), 4096*(c%2)+4096).
Host prep transposes hidden to [H, S_local] per core so the contraction dim (H)
lands on SBUF partitions; w_out is fed in natural [H, O] layout.

Within a core, local token index t maps to SBUF [128, 32] position
(p, j) with t = 32*p + j for the Kuramoto state, and the epilogue
"cs3" rows hold c/s/1 at row element e = t (natural order).
"""
import math
import os

import numpy as np

MIN_K, MAX_K, NUM_STEPS, DT = 0.1, 3.0, 5, 0.01
PI = math.pi

B, S, H = 4, 8192, 1024
N_CORES = 8
TL = S * B // N_CORES  # 4096 tokens per core
NJ = TL // 128         # 32 t-tiles per core
NK = H // 128          # 8 h-tiles (contraction)
INV_S = 1.0 / S

_CACHE = {}

LAST_RESULT = None


def _build(scalars):
    """Build + schedule the Bass program. scalars = (b_phase, b_freq, b_c2)."""
    from concourse import bass, bacc, mybir, tile

    F32 = mybir.dt.float32
    F32R = mybir.dt.float32r
    AF = mybir.ActivationFunctionType
    ALU = mybir.AluOpType

    b_phase, b_freq, b_c2 = scalars

    nc = bacc.Bacc("TRN2", target_bir_lowering=False, debug=False,
                   num_devices=N_CORES)

    # ---- DRAM parameters (per-core shards; host packs these) ----
    hidT_ext = nc.declare_dram_parameter("hidT", [H, TL], F32R, isOutput=False)
    w_ext = nc.declare_dram_parameter("w_main", [H, H], F32R, isOutput=False)
    wpib_ext = nc.declare_dram_parameter("w_pi_b", [4, H], F32R, isOutput=False)
    wpf_ext = nc.declare_dram_parameter("wpf", [H, 2], F32R, isOutput=False)
    w1_ext = nc.declare_dram_parameter("w_c1", [128, 16 * 128], F32, isOutput=False)
    w2_ext = nc.declare_dram_parameter("w_c2", [128, 2], F32, isOutput=False)
    b1t_ext = nc.declare_dram_parameter("b_c1t", [128, 2], F32, isOutput=False)
    ones_ext = nc.declare_dram_parameter("ones", [128, 128], F32, isOutput=False)
    onesr_ext = nc.declare_dram_parameter("ones_row", [1, TL], F32R, isOutput=False)

    out_ext = nc.declare_dram_parameter("out", [TL, H], F32, isOutput=True)
    r_ext = nc.declare_dram_parameter("r_out", [1, 1], F32, isOutput=True)
    k_ext = nc.declare_dram_parameter("K_out", [1, 1], F32, isOutput=True)

    # collective bounce buffers (pack: ph 0:32 | dtf 32:64 | gctx 64:72)
    agin = nc.dram_tensor("agin", [128, 72], F32)
    agout = nc.dram_tensor("agout", [N_CORES * 128, 72], F32, addr_space="Shared")

    with tile.TileContext(nc) as tc:
        with (
            tc.tile_pool(name="res", bufs=1) as res,
            tc.tile_pool(name="psm", bufs=6, space="PSUM") as psm,
            tc.tile_pool(name="pss", bufs=2, space="PSUM") as pss,
        ):
            # ---- resident SBUF tensors ----
            hidT = res.tile([128, NK * TL], F32R, tag="hidT")
            w_sb = res.tile([128, NK * H], F32R, tag="w")
            wpf_sb = res.tile([128, 2 * NK], F32R, tag="wpf")
            w1_sb = res.tile([128, 16 * 128], F32, tag="w1")
            w2_sb = res.tile([128, 2], F32, tag="w2")
            b1t_sb = res.tile([128, 2], F32, tag="b1t")
            ones_sb = res.tile([128, 128], F32, tag="ones")
            wcs3_sb = res.tile([3, H], F32R, tag="wcs3")
            wr_sb = res.tile([1, H], F32R, tag="wr")
            b0_sb = res.tile([1, H], F32R, tag="b0")
            wrb_sb = res.tile([1, H], F32R, tag="wrb")
            cs3_sb = res.tile([3, TL], F32R, tag="cs3")
            ph_sb = res.tile([128, 64], F32, tag="ph")
            dtf_sb = res.tile([128, 64], F32, tag="dtf")
            csh_sb = res.tile([128, 64], F32, tag="csh")
            cs_sb = res.tile([128, 128], F32R, tag="cs")
            u1_sb = res.tile([128, 64], F32, tag="u1")
            u2_sb = res.tile([128, 64], F32, tag="u2")
            rsum_sb = res.tile([128, 2], F32, tag="rsum")
            gct_sb = res.tile([128, NK], F32, tag="gct")
            gtmp_sb = res.tile([128, 64], F32, tag="gtmp")
            ptn_sb = res.tile([128, 72], F32, tag="ptn")
            hmid_sb = res.tile([128, 2], F32, tag="hmid")
            small_sb = res.tile([128, 8], F32, tag="small")
            # small_sb cols: 0=r_col, 1=ndtkn_col; row0: 2=sig, 3=K, 4=ndtkn
            # cols 5,6: scratch for r

            # ---- hidden DMAs first (j4-outer, k-inner); w after first block ----
            for j4 in range(8):
                for k in range(NK):
                    nc.sync.dma_start(
                        hidT[:, k * TL + 512 * j4:k * TL + 512 * j4 + 512],
                        hidT_ext[128 * k:128 * k + 128, 512 * j4:512 * j4 + 512],
                    )
                if j4 == 0:
                    for k in range(NK):
                        nc.sync.dma_start(
                            w_sb[:, k * H:(k + 1) * H],
                            w_ext[128 * k:128 * k + 128, :],
                        )

            # ---- small constant DMAs ----
            nc.sync.dma_start(
                wpf_sb[:],
                wpf_ext[:].rearrange("(k p) c -> p k c", k=NK, p=128),
            )
            nc.sync.dma_start(w1_sb[:], w1_ext[:])
            nc.sync.dma_start(w2_sb[:], w2_ext[:])
            nc.sync.dma_start(b1t_sb[:], b1t_ext[:])
            nc.sync.dma_start(ones_sb[:], ones_ext[:])
            nc.sync.dma_start(wr_sb[:], wpib_ext[2:3, :])
            nc.sync.dma_start(b0_sb[:], wpib_ext[3:4, :])
            nc.sync.dma_start(wcs3_sb[0:2, :], wpib_ext[0:2, :])
            nc.sync.dma_start(cs3_sb[2:3, :], onesr_ext[:])

            # ---- phase A: p_raw/f_raw rows via w-stationary matmuls ----
            # psum [2, 512] per 512-token chunk; ACT-copy into cs3 rows 0/1
            # (scratch reuse), then shuffle to [128, 32] (t = 32p + j).
            for tc8 in range(8):
                pf_ps = pss.tile([2, 512], F32, tag="pf")
                for k in range(NK):
                    nc.tensor.matmul(
                        pf_ps[:],
                        wpf_sb[:, 2 * k:2 * k + 2],
                        hidT[:, k * TL + 512 * tc8:k * TL + 512 * tc8 + 512],
                        start=(k == 0), stop=(k == NK - 1),
                    )
                with nc.allow_low_precision(reason="f32r pf rows"):
                    nc.scalar.activation(
                        cs3_sb[0:2, 512 * tc8:512 * tc8 + 512],
                        pf_ps[:], AF.Copy)
                # gctx partials for this chunk column range
                for k in range(NK):
                    nc.vector.tensor_reduce(
                        out=gtmp_sb[:, 8 * k + tc8:8 * k + tc8 + 1],
                        in_=hidT[:, k * TL + 512 * tc8:k * TL + 512 * tc8 + 512].bitcast(F32),
                        axis=mybir.AxisListType.X, op=ALU.add,
                    )
            for k in range(NK):
                nc.vector.tensor_reduce(
                    out=gct_sb[:, k:k + 1],
                    in_=gtmp_sb[:, 8 * k:8 * k + 8],
                    axis=mybir.AxisListType.X, op=ALU.add,
                )
            # shuffle p_raw/f_raw rows -> [128, 32] (contiguous: e = 32p + j)
            nc.sync.dma_start(
                ph_sb[:, 0:32],
                cs3_sb[0:1, :].bitcast(F32).rearrange("a (p j) -> a p j", p=128, j=32),
            )
            nc.sync.dma_start(
                dtf_sb[:, 0:32],
                cs3_sb[1:2, :].bitcast(F32).rearrange("a (p j) -> a p j", p=128, j=32),
            )
            # ph = pi * tanh(p_raw + b_phase) ; dtf = DT*(f_raw + b_freq)
            nc.scalar.activation(ph_sb[:, 0:32], ph_sb[:, 0:32], AF.Tanh,
                                 bias=float(b_phase), scale=1.0)
            nc.vector.tensor_single_scalar(
                out=ph_sb[:, 0:32], in_=ph_sb[:, 0:32], scalar=PI, op=ALU.mult)
            nc.vector.tensor_scalar(
                out=dtf_sb[:, 0:32], in0=dtf_sb[:, 0:32],
                scalar1=DT, scalar2=DT * float(b_freq),
                op0=ALU.mult, op1=ALU.add)

            # ---- AllGather: share (ph, dtf, gctx) with partner ----
            nc.sync.dma_start(agin[:, 0:32], ph_sb[:, 0:32])
            nc.sync.dma_start(agin[:, 32:64], dtf_sb[:, 0:32])
            nc.sync.dma_start(agin[:, 64:72], gct_sb[:])
            nc.gpsimd.collective_compute(
                "AllGather", ALU.bypass,
                ins=[agin[:].opt()], outs=[agout[:].opt()],
                replica_groups=[list(range(N_CORES))],
            )
            pid = nc.sync.partition_id()
            for c in range(N_CORES):
                t = c ^ 1
                is_partner = (pid >= t) & (pid <= t)
                nc.sync.dma_start(ptn_sb[:], agout[128 * c:128 * c + 128, :],
                                  cond=is_partner)
            nc.vector.tensor_copy(ph_sb[:, 32:64], ptn_sb[:, 0:32])
            nc.vector.tensor_copy(dtf_sb[:, 32:64], ptn_sb[:, 32:64])
            nc.vector.tensor_tensor(out=gct_sb[:], in0=gct_sb[:],
                                    in1=ptn_sb[:, 64:72], op=ALU.add)

            # ---- K MLP: gctx -> gelu(W1) -> sigmoid(W2) -> K ----
            hmid_ps = pss.tile([128, 2], F32, tag="pf")
            for q in range(2):
                for k in range(NK):
                    nc.tensor.matmul(
                        hmid_ps[:, q:q + 1],
                        w1_sb[:, (2 * k + q) * 128:(2 * k + q) * 128 + 128],
                        gct_sb[:, k:k + 1],
                        start=(k == 0), stop=(k == NK - 1),
                    )
            for q in range(2):
                nc.scalar.activation(hmid_sb[:, q:q + 1], hmid_ps[:, q:q + 1],
                                     AF.Gelu, bias=b1t_sb[:, q:q + 1], scale=INV_S)
            kp_ps = pss.tile([1, 1], F32, tag="pf")
            for q in range(2):
                nc.tensor.matmul(
                    kp_ps[:],
                    hmid_sb[:, q:q + 1],
                    w2_sb[:, q:q + 1],
                    start=(q == 0), stop=(q == 1),
                )
            nc.scalar.activation(small_sb[0:1, 2:3], kp_ps[:], AF.Sigmoid,
                                 bias=float(b_c2), scale=1.0)
            # K = MIN_K + (MAX_K-MIN_K)*sig ; ndtkn = -DT*K/S
            nc.vector.tensor_scalar(
                out=small_sb[0:1, 3:4], in0=small_sb[0:1, 2:3],
                scalar1=MAX_K - MIN_K, scalar2=MIN_K, op0=ALU.mult, op1=ALU.add)
            nc.vector.tensor_scalar(
                out=small_sb[0:1, 4:5], in0=small_sb[0:1, 2:3],
                scalar1=-DT * (MAX_K - MIN_K) * INV_S,
                scalar2=-DT * MIN_K * INV_S, op0=ALU.mult, op1=ALU.add)
            nc.sync.dma_start(k_ext[:], small_sb[0:1, 3:4])
            # replicate ndtkn to all partitions: ones[1,128].T @ ndtkn[1,1]
            rep_ps = pss.tile([128, 1], F32, tag="pf")
            nc.tensor.matmul(rep_ps[:], ones_sb[0:1, 0:128], small_sb[0:1, 4:5],
                             start=True, stop=True)
            nc.vector.tensor_copy(small_sb[:, 1:2], rep_ps[:])

            # ---- Kuramoto steps on [128, 64] (8192 tokens of this batch) ----
            for step in range(NUM_STEPS + 1):
                last = step == NUM_STEPS
                nc.vector.add_range_wrap(csh_sb[:], ph_sb[:], PI / 2, PI, 2 * PI)
                with nc.allow_low_precision(reason="f32r cs for matmul epilogue"):
                    nc.scalar.activation(cs_sb[:, 0:64], csh_sb[:], AF.Sin,
                                         accum_out=rsum_sb[:, 0:1])
                    nc.scalar.activation(cs_sb[:, 64:128], ph_sb[:], AF.Sin,
                                         accum_out=rsum_sb[:, 1:2])
                sums_ps = pss.tile([128, 2], F32, tag="pf")
                nc.tensor.matmul(sums_ps[:], ones_sb[:], rsum_sb[:],
                                 start=True, stop=True)
                if not last:
                    # u1 = s*C ; u2 = c*S - u1 = -interaction
                    nc.vector.tensor_single_scalar(
                        out=u1_sb[:], in_=cs_sb[:, 64:128].bitcast(F32),
                        scalar=sums_ps[:, 0:1], op=ALU.mult)
                    nc.vector.scalar_tensor_tensor(
                        out=u2_sb[:], in0=cs_sb[:, 0:64].bitcast(F32),
                        scalar=sums_ps[:, 1:2], in1=u1_sb[:],
                        op0=ALU.mult, op1=ALU.subtract)
                    # ph = wrap(ph + dtf + ndtkn*u2)
                    nc.vector.tensor_tensor(out=ph_sb[:], in0=ph_sb[:],
                                            in1=dtf_sb[:], op=ALU.add)
                    nc.vector.scalar_tensor_tensor(
                        out=ph_sb[:], in0=u2_sb[:], scalar=small_sb[:, 1:2],
                        in1=ph_sb[:], op0=ALU.mult, op1=ALU.add)
                    nc.vector.add_range_wrap(ph_sb[:], ph_sb[:], 0.0, PI, 2 * PI)
                else:
                    # r = sqrt((Csum/S)^2 + (Ssum/S)^2) replicated
                    nc.vector.tensor_single_scalar(
                        out=small_sb[:, 5:6], in_=sums_ps[:, 1:2],
                        scalar=sums_ps[:, 1:2], op=ALU.mult)
                    nc.vector.scalar_tensor_tensor(
                        out=small_sb[:, 6:7], in0=sums_ps[:, 0:1],
                        scalar=sums_ps[:, 0:1], in1=small_sb[:, 5:6],
                        op0=ALU.mult, op1=ALU.add)
                    nc.scalar.activation(small_sb[:, 0:1], small_sb[:, 6:7],
                                         AF.Sqrt, bias=0.0, scale=INV_S * INV_S)
                    nc.sync.dma_start(r_ext[:], small_sb[0:1, 0:1])

            # ---- build cs3 rows: c_own/s_own as [1, 4096] (e = t = 32p + j) ----
            nc.sync.dma_start(
                cs3_sb[0:1, :].rearrange("a (p j) -> a p j", p=128, j=32),
                cs_sb[:, 0:32],
            )
            nc.sync.dma_start(
                cs3_sb[1:2, :].rearrange("a (p j) -> a p j", p=128, j=32),
                cs_sb[:, 64:96],
            )
            # wrb = r*wr + b_out (partition 0) -> wcs3 row 2
            with nc.allow_low_precision(reason="f32r epilogue row"):
                nc.vector.scalar_tensor_tensor(
                    out=wrb_sb[0:1, :], in0=wr_sb[0:1, :].bitcast(F32),
                    scalar=small_sb[0:1, 0:1], in1=b0_sb[0:1, :].bitcast(F32),
                    op0=ALU.mult, op1=ALU.add)
            nc.sync.dma_start(wcs3_sb[2:3, :], wrb_sb[0:1, :])

            # hidT strip view: [128, k(NK), j(NJ), t(128)]
            hid4 = hidT[:].rearrange(
                "p (k j t) -> p k j t", k=NK, j=NJ, t=128)

            def staged(j, half):
                # 4 strips k = 4*half .. 4*half+3 of tile column j
                lo = 4 * half
                return hid4[:, lo:lo + 4, j:j + 1, :]

            # ---- main matmul: per (j, half) 8-MM group, staged into the ----
            # ---- hidT column strips freed by consuming tile j           ----
            for j in range(NJ):
                ps_halves = []
                for half in range(2):
                    o0 = 512 * half
                    mm_ps = psm.tile([128, 512], F32, tag="mm")
                    for k in range(NK):
                        nc.tensor.matmul(
                            mm_ps[:],
                            hidT[:, k * TL + 128 * j:k * TL + 128 * j + 128],
                            w_sb[:, k * H + o0:k * H + o0 + 512],
                            start=(k == 0), stop=(k == NK - 1),
                        )
                    ps_halves.append(mm_ps)
                # stage only after BOTH halves consumed tile j's hidT strips
                for half in range(2):
                    with nc.allow_low_precision(reason="f32r staging"):
                        nc.any.tensor_copy(staged(j, half), ps_halves[half][:])

            # ---- phase epilogue: K=3 matmul + add, then DMA out ----
            for j in range(NJ):
                for half in range(2):
                    o0 = 512 * half
                    k3_ps = psm.tile([128, 512], F32, tag="mm")
                    nc.tensor.matmul(
                        k3_ps[:],
                        cs3_sb[:, 128 * j:128 * j + 128],
                        wcs3_sb[:, o0:o0 + 512],
                        start=True, stop=True,
                    )
                    with nc.allow_low_precision(reason="f32r staging add"):
                        nc.vector.tensor_tensor(
                            out=staged(j, half), in0=staged(j, half).bitcast(F32),
                            in1=k3_ps[:], op=ALU.add)
                nc.sync.dma_start(
                    out_ext[128 * j:128 * j + 128, :],
                    hid4[:, 0:NK, j:j + 1, :].bitcast(F32),
                )

    nc.compile()
    return nc


def _get_nc(scalars):
    key = tuple(float(x) for x in scalars)
    if key not in _CACHE:
        _CACHE[key] = _build(key)
    return _CACHE[key]


def kernel(hidden_states, w_c1, b_c1, w_c2, b_c2, w_phase, b_phase,
           w_freq, b_freq, w_out, b_out):
    global LAST_RESULT
    from concourse.bass_utils import run_bass_kernel_spmd

    hidden_states = np.asarray(hidden_states, dtype=np.float32)
    w_c1 = np.asarray(w_c1, dtype=np.float32)
    b_c1 = np.asarray(b_c1, dtype=np.float32)
    w_c2 = np.asarray(w_c2, dtype=np.float32)
    b_c2 = np.asarray(b_c2, dtype=np.float32)
    w_phase = np.asarray(w_phase, dtype=np.float32)
    b_phase = np.asarray(b_phase, dtype=np.float32)
    w_freq = np.asarray(w_freq, dtype=np.float32)
    b_freq = np.asarray(b_freq, dtype=np.float32)
    w_out = np.asarray(w_out, dtype=np.float32)
    b_out = np.asarray(b_out, dtype=np.float32)

    nc = _get_nc((float(b_phase[0]), float(b_freq[0]), float(b_c2[0])))

    # ---- host-side packing (sharding + layout) ----
    w_main = np.ascontiguousarray(w_out[0:H, :])                       # [H, H]
    w_pi_b = np.ascontiguousarray(
        np.concatenate([w_out[H:H + 3, :], b_out[None, :]], axis=0))   # [4, H]
    wpf = np.ascontiguousarray(
        np.concatenate([w_phase, w_freq], axis=1))                     # [H, 2]
    w1 = np.ascontiguousarray(
        w_c1.reshape(NK, 128, 2, 128).transpose(1, 0, 2, 3).reshape(128, 16 * 128))
    w2 = np.ascontiguousarray(w_c2.reshape(2, 128, 1)[:, :, 0].T)      # [128, 2]
    b1t = np.ascontiguousarray(b_c1.reshape(2, 128).T)                 # [128, 2]
    ones = np.ones((128, 128), dtype=np.float32)
    ones_row = np.ones((1, TL), dtype=np.float32)

    in_maps = []
    for c in range(N_CORES):
        b = c // 2
        s0 = TL * (c % 2)
        hidT = np.ascontiguousarray(hidden_states[b, s0:s0 + TL, :].T)  # [H, TL]
        in_maps.append({
            "hidT": hidT, "w_main": w_main, "w_pi_b": w_pi_b, "wpf": wpf,
            "w_c1": w1, "w_c2": w2, "b_c1t": b1t, "ones": ones,
            "ones_row": ones_row,
        })

    res = run_bass_kernel_spmd(nc, in_maps, list(range(N_CORES)),
                               trace=os.environ.get("BASS_TRACE", "0") == "1")
    LAST_RESULT = res

    output = np.empty((B, S, H), dtype=np.float32)
    r = np.empty((B,), dtype=np.float32)
    K = np.empty((B,), dtype=np.float32)
    for c in range(N_CORES):
        b = c // 2
        s0 = TL * (c % 2)
        output[b, s0:s0 + TL, :] = res.results[c]["out"]
        if c % 2 == 0:
            r[b] = res.results[c]["r_out"][0, 0]
            K[b] = res.results[c]["K_out"][0, 0]
    return output, r, K
